# revision 13
# baseline (speedup 1.0000x reference)
"""Trainium2 Bass kernel for a ResNet Bottleneck block (inference).

Reference computation (NCHW, N=128, Cin=Cout=1024, width=256, H=W=14):
    out = relu(bn1(conv1x1(x, w1)))          # 1024 -> 256
    out = relu(bn2(conv3x3(out, w2, pad=1))) # 256 -> 256
    out = bn3(conv1x1(out, w3))              # 256 -> 1024
    y   = relu(out + x)

Strategy (fp8 DoubleRow):
- Data-parallel: batch 128 sharded as 16 images per NeuronCore (8 cores).
- All convs run as fp8e4 (e4m3) DoubleRow matmuls: 2 fp8 weights/cell double
  the effective contraction to 256/matmul (~1.5x bf16 TFLOP/s at free-dim
  >=196). PSUM accumulates fp32, so precision loss is only operand
  quantization; measured end-to-end rel err ~8e-3 (tol 2e-2).
- BN folded on host into weight scale + bias. Weights are rescaled by
  powers of two (s1=32, s2=2, s3=16) to lift their ~0.02 std out of
  e4m3's subnormal range; ReLU's positive homogeneity carries the scale
  through layers, evictions add correspondingly scaled biases, and the
  host divides the final bf16 output by s1*s2*s3 = 1024.
- conv2 (3x3, pad 1) uses a zero-padded 16x16 per-image SBUF layout; each
  of the 9 taps is one shifted-window DoubleRow matmul per image (moving
  AP [p, ktile=2, row14, col14]; matmul APs allow at most 3 free dims,
  so images can't be paired here).
- Residual + bias3 are folded host-side into xr = 1024*(x + b3), bf16.
  Half the conv3 groups add it on the PE (bf16 identity matmul appended
  to the fp8 accumulation group), half on DVE (tensor_tensor add) with
  the ReLU on ACT, balancing PE vs eviction-engine load.
- conv2+conv3 are pipelined per 4-image "super" block so conv3 evictions
  overlap the next block's conv2 matmuls.
"""

import sys

if "/opt/trn_rl_repo" not in sys.path:
    sys.path.insert(0, "/opt/trn_rl_repo")

import numpy as np
import ml_dtypes

import concourse.bass as bass
import concourse.bacc as bacc
import concourse.tile as tile
from concourse import mybir
from concourse.bass_utils import run_bass_kernel_spmd

EPS = 1e-5
NCORES = 8
NLOC = 16          # images per core
P = 128
C_IN = 1024
WIDTH = 256
C_OUT = 1024
HW = 196           # 14*14
JB = 4             # conv1 contraction double-blocks (1024 = 4*256)
MB3 = 8            # conv3 output 128-blocks
NPAIRS = 8         # image pairs per core
NF = 2 * HW        # 392

S1, S2, S3 = 32.0, 2.0, 16.0
STOT = S1 * S2 * S3            # 1024; fp8 activation scales: out1 32x, out2 64x

BF16 = mybir.dt.bfloat16
F32 = mybir.dt.float32
FP8 = mybir.dt.float8e4
DR = mybir.MatmulPerfMode.DoubleRow
Relu = mybir.ActivationFunctionType.Relu

_cached = {}


def _build():
    """Build + compile the SPMD NEFF (one core's program). Cached."""
    if "nc" in _cached:
        return _cached["nc"]

    nc = bacc.Bacc("TRN2", target_bir_lowering=False, debug=False,
                   num_devices=NCORES)

    # DRAM layouts are exact SBUF images (partition-major), packed host-side.
    x8_d = nc.dram_tensor("x8", [P, JB * 2 * NLOC * HW], FP8,
                          kind="ExternalInput")
    xr_d = nc.dram_tensor("xr", [NPAIRS, P, MB3 * NF], BF16,
                          kind="ExternalInput")
    w1_d = nc.dram_tensor("w1t", [P, JB * 2 * WIDTH], FP8,
                          kind="ExternalInput")
    w2_d = nc.dram_tensor("w2t", [P, 9 * 2 * WIDTH], FP8,
                          kind="ExternalInput")
    w3_d = nc.dram_tensor("w3t", [P, 2 * C_OUT], FP8, kind="ExternalInput")
    b_d = nc.dram_tensor("biases", [P, 4], F32, kind="ExternalInput")
    id_d = nc.dram_tensor("ident", [P, P], BF16, kind="ExternalInput")
    y_d = nc.dram_tensor("y", [NPAIRS, P, MB3 * NF], BF16,
                         kind="ExternalOutput")

    with tile.TileContext(nc) as tc:
        _emit(tc, nc, x8_d, xr_d, w1_d, w2_d, w3_d, b_d, id_d, y_d)

    nc.compile()
    _cached["nc"] = nc
    return nc


def _emit(tc, nc, x8_d, xr_d, w1_d, w2_d, w3_d, b_d, id_d, y_d):
    import contextlib

    Alu = mybir.AluOpType

    with contextlib.ExitStack() as ctx:
        const = ctx.enter_context(tc.tile_pool(name="const", bufs=1))
        xpool = ctx.enter_context(tc.tile_pool(name="xpool", bufs=1))
        opool = ctx.enter_context(tc.tile_pool(name="opool", bufs=1))
        # PSUM as 4 pair-tiles of [P, 2, 512] f32 (2 banks each = all 8
        # banks): two accumulation groups per tile in separate zero
        # regions, letting bias-free eviction pairs merge into one op.
        psp = ctx.enter_context(tc.tile_pool(name="psp", bufs=4, space="PSUM"))
        evp = ctx.enter_context(tc.tile_pool(name="evp", bufs=2))

        # ---- Loads -----------------------------------------------------
        # All bulk transfers as 2D [P, contiguous] DMAs on the two HWDGE
        # rings (sync + scalar): SWDGE (gpsimd) descriptor generation is a
        # software loop that delays first-byte by many microseconds.
        # Within a ring, DMAs execute FIFO. x8's (j, kt) blocks are striped
        # pairwise across both rings so each conv1 contraction double-block
        # (kt pair) lands complete as early as possible; weights w2/w3/id
        # queue behind the x8 blocks on the scalar ring (not needed until
        # conv2), w1+biases lead it (needed by the first matmul).
        x8sb = xpool.tile([P, JB, 2, NLOC * HW], FP8, name="x8sb", tag="x8sb")
        x8v = x8sb[:].rearrange("p j k f -> p (j k) f")
        x8src = x8_d.ap().rearrange("p (b f) -> p b f", b=2 * JB)

        w1sb = const.tile([P, JB, 2, WIDTH], FP8, name="w1sb", tag="w1sb")
        nc.scalar.dma_start(w1sb[:].rearrange("p a k c -> p (a k c)"),
                            w1_d.ap())

        ball = const.tile([P, 4], F32, name="ball", tag="ball")
        nc.scalar.dma_start(ball[:], b_d.ap())

        for b in range(2 * JB):
            eng = nc.sync if b % 2 == 0 else nc.scalar
            eng.dma_start(x8v[:, b, :], x8src[:, b, :])

        xrsb = xpool.tile([P, NPAIRS, MB3, NF], BF16, name="xrsb", tag="xrsb")
        for np_ in range(NPAIRS):
            dst = xrsb[:, np_, :, :].rearrange("p m f -> p (m f)")
            nc.sync.dma_start(dst, xr_d.ap()[np_])

        w2sb = const.tile([P, 9, 2, WIDTH], FP8, name="w2sb", tag="w2sb")
        nc.scalar.dma_start(w2sb[:].rearrange("p t k c -> p (t k c)"),
                            w2_d.ap())

        w3sb = const.tile([P, 2, C_OUT], FP8, name="w3sb", tag="w3sb")
        nc.scalar.dma_start(w3sb[:].rearrange("p k c -> p (k c)"),
                            w3_d.ap())

        id_t = const.tile([P, P], BF16, name="id_t", tag="id_t")
        nc.scalar.dma_start(id_t[:], id_d.ap())

        # PE warm-up: bridge PE activity from body-start until the first x8
        # block pair lands (HAM keeps warming through conv1's own matmuls).
        scratch = const.tile([P, 512], BF16, name="scratch", tag="scratch")
        nc.gpsimd.memset(scratch[:], 0.0)
        warm_ps = psp.tile([P, 2, 512], F32, name="warm_ps", tag="pp")
        for _ in range(3):
            nc.tensor.matmul(warm_ps[:, 0, :], scratch[:, 0:P], scratch[:],
                             start=True, stop=True)

        # Zero-padded conv1 output: per image a 16x16 field per 128-block,
        # payload at rows/cols 1..14. Border zeroing on the otherwise-idle
        # GpSimd so DVE stays free for evictions.
        out1 = opool.tile([P, 2, NLOC, 16, 16], FP8, name="out1", tag="out1")
        o1flat = out1[:].rearrange("p k i r c -> p k (i r c)")
        for half in range(2):
            nc.gpsimd.memset(o1flat[:, :, half * 2048:(half + 1) * 2048], 0.0)

        out2 = opool.tile([P, 2, NLOC * HW], FP8, name="out2", tag="out2")

        tog = [0]

        def evict_relu_bias(dst, src, bias_ap):
            # dst = relu(src + bias), alternating DVE / ACT
            tog[0] ^= 1
            if tog[0]:
                nc.vector.tensor_scalar(dst, src, bias_ap, 0.0, Alu.add,
                                        Alu.max)
            else:
                nc.scalar.activation(dst, src, Relu, bias=bias_ap)

        # ---- conv1 (1x1, 1024->256) + bias + relu -> padded out1 --------
        # Per half: 8 open groups (4 pair-tiles x 2 out-blocks),
        # contraction j outer, groups inner so consecutive matmuls hit
        # different banks.
        for half in range(2):
            nls = range(4)
            pair = {nl: psp.tile([P, 2, 512], F32, name=f"ps1_{nl}",
                                 tag="pp") for nl in nls}
            for j in range(JB):
                for mo in range(2):
                    w_ap = w1sb[:, j, :, mo * P:(mo + 1) * P]
                    for nl in nls:
                        np_ = 4 * half + nl
                        nc.tensor.matmul(
                            pair[nl][:, mo, 0:NF], w_ap,
                            x8sb[:, j, :, np_ * NF:(np_ + 1) * NF],
                            start=(j == 0), stop=(j == JB - 1),
                            perf_mode=DR)
            for nl in nls:
                np_ = 4 * half + nl
                for mo in range(2):
                    dst = out1[:, mo, 2 * np_:2 * np_ + 2, 1:15, 1:15]
                    src = (pair[nl][:, mo, 0:NF]
                           .rearrange("p (i r c) -> p i r c", i=2, r=14))
                    evict_relu_bias(dst, src, ball[:, mo:mo + 1])

        # ---- conv2 + conv3 pipelined per 4-image super-block ------------
        for s in range(4):
            # conv2 (3x3, 256->256, pad 1): 8 groups (4 img pair-tiles x 2
            # out-blocks), contraction tap outer. Per-image matmuls (N=196):
            # the windowed moving AP [p, kt, r, c] is at the 3-free-dim ISA
            # limit. Evictions stay per-group (bias differs per out-block).
            p2 = {ii: psp.tile([P, 2, 512], F32, name=f"ps2_{ii}", tag="pp")
                  for ii in range(4)}
            for tap in range(9):
                dy, dx = tap // 3, tap % 3
                for mo in range(2):
                    w_ap = w2sb[:, tap, :, mo * P:(mo + 1) * P]
                    for ii in range(4):
                        img = 4 * s + ii
                        nc.tensor.matmul(
                            p2[ii][:, mo, 0:HW].rearrange("p (r c) -> p r c",
                                                          r=14),
                            w_ap,
                            out1[:, :, img, dy:dy + 14, dx:dx + 14],
                            start=(tap == 0), stop=(tap == 8),
                            perf_mode=DR)
            for ii in range(4):
                img = 4 * s + ii
                for mo in range(2):
                    evict_relu_bias(out2[:, mo, img * HW:(img + 1) * HW],
                                    p2[ii][:, mo, 0:HW], ball[:, 2 + mo:3 + mo])

            # conv3 (1x1, 256->1024) + residual + relu, two waves of 8
            # groups (4 m-blocks x 2 pairs). The residual (with bias3
            # pre-folded and pre-scaled, bf16) enters every group as an
            # identity matmul on the PE, so each eviction is a single
            # relu-cast op: eviction engines run ~0.56us per [P,392] op,
            # and halving their op count is worth more than the ~165ns/MM
            # the identity adds to the PE.
            yst = {nl: evp.tile([P, MB3 * NF], BF16, name=f"yst{nl}",
                                tag="yst", bufs=3) for nl in range(2)}
            for wave in range(2):
                p3 = {(mp, nl): psp.tile([P, 2, 512], F32,
                                         name=f"ps3_{mp}_{nl}", tag="pp")
                      for mp in range(2) for nl in range(2)}
                for mi in range(4):
                    m = 4 * wave + mi
                    w_ap = w3sb[:, :, m * P:(m + 1) * P]
                    for nl in range(2):
                        np_ = 2 * s + nl
                        nc.tensor.matmul(
                            p3[(mi // 2, nl)][:, mi % 2, 0:NF], w_ap,
                            out2[:, :, np_ * NF:(np_ + 1) * NF],
                            start=True, stop=False, perf_mode=DR)
                for mi in range(4):
                    m = 4 * wave + mi
                    for nl in range(2):
                        np_ = 2 * s + nl
                        nc.tensor.matmul(
                            p3[(mi // 2, nl)][:, mi % 2, 0:NF], id_t[:],
                            xrsb[:, np_, m, :], start=False, stop=True)
                # merged eviction: one relu-cast per pair-tile (bias3 is
                # already in the residual), writing two adjacent m-blocks
                for mp in range(2):
                    m0 = 4 * wave + 2 * mp
                    for nl in range(2):
                        dst = (yst[nl][:, m0 * NF:(m0 + 2) * NF]
                               .rearrange("p (g f) -> p g f", g=2))
                        src = p3[(mp, nl)][:, :, 0:NF]
                        tog[0] ^= 1
                        if tog[0]:
                            nc.vector.tensor_scalar_max(dst, src, 0.0)
                        else:
                            nc.scalar.activation(dst, src, Relu, bias=0.0)
                # per-wave y halves overlap the next wave's compute
                for nl in range(2):
                    np_ = 2 * s + nl
                    h0, h1 = 4 * wave * NF, (4 * wave + 4) * NF
                    nc.sync.dma_start(y_d.ap()[np_][:, h0:h1],
                                      yst[nl][:, h0:h1])


def _prep(x, w1, g1, b1, m1, v1, w2, g2, b2, m2, v2, w3, g3, b3, m3, v3):
    """Host-side: fold BN, rescale + quantize to fp8, pack SBUF images."""
    def fold(w, g, b, m, v):
        scale = (g.astype(np.float64) / np.sqrt(v.astype(np.float64) + EPS))
        bias = b.astype(np.float64) - m.astype(np.float64) * scale
        wf = w.astype(np.float64) * scale.reshape(-1, *([1] * (w.ndim - 1)))
        return wf.astype(np.float32), bias.astype(np.float32)

    w1f, bias1 = fold(w1, g1, b1, m1, v1)   # [256,1024,1,1]
    w2f, bias2 = fold(w2, g2, b2, m2, v2)   # [256,256,3,3]
    w3f, bias3 = fold(w3, g3, b3, m3, v3)   # [1024,256,1,1]

    bf = ml_dtypes.bfloat16
    e4 = ml_dtypes.float8_e4m3

    def q8(a):
        return np.clip(a, -240.0, 240.0).astype(e4)

    # lhsT SBUF images [p_in, ..., ktile, co]:
    w1t = q8(np.ascontiguousarray(
        (w1f[:, :, 0, 0] * S1).T.reshape(JB, 2, P, WIDTH)
        .transpose(2, 0, 1, 3).reshape(P, JB * 2 * WIDTH)))
    w2t = q8(np.ascontiguousarray(
        (w2f * S2).transpose(2, 3, 1, 0).reshape(3, 3, 2, P, WIDTH)
        .transpose(3, 0, 1, 2, 4).reshape(P, 9 * 2 * WIDTH)))
    w3t = q8(np.ascontiguousarray(
        (w3f[:, :, 0, 0] * S3).T.reshape(2, P, C_OUT)
        .transpose(1, 0, 2).reshape(P, 2 * C_OUT)))

    b1h = (bias1 * S1).reshape(2, P).T                    # [P, 2]
    b2h = (bias2 * S1 * S2).reshape(2, P).T               # [P, 2] (64x)
    ball = np.ascontiguousarray(
        np.concatenate([b1h, b2h], axis=1), dtype=np.float32)

    # x8: conv1 moving operand, [core][P, (j, kt, img16, hw)] fp8
    xs = (x.reshape(NCORES, NLOC, JB, 2, P, HW)
          .transpose(0, 4, 2, 3, 1, 5)
          .reshape(NCORES, P, JB * 2 * NLOC * HW))
    x8 = q8(xs)

    # xr: residual + bias3, pre-scaled: STOT*(x + b3), np-major bf16
    r = x.reshape(NCORES, NLOC, C_OUT, HW) + bias3[None, None, :, None]
    xr = ((r * STOT)
          .reshape(NCORES, NPAIRS, 2, MB3, P, HW)
          .transpose(0, 1, 4, 3, 2, 5)
          .reshape(NCORES, NPAIRS, P, MB3 * NF)).astype(bf)

    common = {"w1t": w1t, "w2t": w2t, "w3t": w3t, "biases": ball,
              "ident": np.eye(P, dtype=np.float32).astype(bf)}
    in_maps = [dict(common, x8=np.ascontiguousarray(x8[i]),
                    xr=np.ascontiguousarray(xr[i]))
               for i in range(NCORES)]
    return in_maps


def kernel(**inputs):
    inputs = {k: np.asarray(v) for k, v in inputs.items()}
    in_maps = _prep(**inputs)
    nc = _build()
    res = run_bass_kernel_spmd(nc, in_maps, core_ids=list(range(NCORES)))

    y = np.empty((NCORES * NLOC, C_OUT, 14, 14), dtype=np.float32)
    for i in range(NCORES):
        r = np.asarray(res.results[i]["y"], dtype=np.float32) / STOT
        r = (r.reshape(NPAIRS, P, MB3, 2, HW)
             .transpose(0, 3, 2, 1, 4)
             .reshape(NLOC, C_OUT, 14, 14))
        y[i * NLOC:(i + 1) * NLOC] = r
    return y


# revision 17
# speedup vs baseline: 1.0418x; 1.0418x over previous
"""Trainium2 Bass kernel for a ResNet Bottleneck block (inference).

Reference computation (NCHW, N=128, Cin=Cout=1024, width=256, H=W=14):
    out = relu(bn1(conv1x1(x, w1)))          # 1024 -> 256
    out = relu(bn2(conv3x3(out, w2, pad=1))) # 256 -> 256
    out = bn3(conv1x1(out, w3))              # 256 -> 1024
    y   = relu(out + x)

Strategy (fp8 DoubleRow):
- Data-parallel: batch 128 sharded as 16 images per NeuronCore (8 cores).
- All convs run as fp8e4 (e4m3) DoubleRow matmuls: 2 fp8 weights/cell double
  the effective contraction to 256/matmul (~1.5x bf16 TFLOP/s at free-dim
  >=196). PSUM accumulates fp32, so precision loss is only operand
  quantization; measured end-to-end rel err ~8e-3 (tol 2e-2).
- BN folded on host into weight scale + bias. Weights are rescaled by
  powers of two (s1=32, s2=2, s3=16) to lift their ~0.02 std out of
  e4m3's subnormal range; ReLU's positive homogeneity carries the scale
  through layers, evictions add correspondingly scaled biases, and the
  host divides the final bf16 output by s1*s2*s3 = 1024.
- conv2 (3x3, pad 1) uses a zero-padded 16x16 per-image SBUF layout; each
  of the 9 taps is one shifted-window DoubleRow matmul per image (moving
  AP [p, ktile=2, row14, col14]; matmul APs allow at most 3 free dims,
  so images can't be paired here).
- Residual + bias3 are folded host-side into xr = 1024*(x + b3), bf16.
  Half the conv3 groups add it on the PE (bf16 identity matmul appended
  to the fp8 accumulation group), half on DVE (tensor_tensor add) with
  the ReLU on ACT, balancing PE vs eviction-engine load.
- conv2+conv3 are pipelined per 4-image "super" block so conv3 evictions
  overlap the next block's conv2 matmuls.
"""

import sys

if "/opt/trn_rl_repo" not in sys.path:
    sys.path.insert(0, "/opt/trn_rl_repo")

import numpy as np
import ml_dtypes

import concourse.bass as bass
import concourse.bacc as bacc
import concourse.tile as tile
from concourse import mybir
from concourse.bass_utils import run_bass_kernel_spmd

EPS = 1e-5
NCORES = 8
NLOC = 16          # images per core
P = 128
C_IN = 1024
WIDTH = 256
C_OUT = 1024
HW = 196           # 14*14
JB = 4             # conv1 contraction double-blocks (1024 = 4*256)
MB3 = 8            # conv3 output 128-blocks
NPAIRS = 8         # image pairs per core
NF = 2 * HW        # 392

S1, S2, S3 = 32.0, 2.0, 16.0
STOT = S1 * S2 * S3            # 1024; fp8 activation scales: out1 32x, out2 64x

BF16 = mybir.dt.bfloat16
F32 = mybir.dt.float32
FP8 = mybir.dt.float8e4
DR = mybir.MatmulPerfMode.DoubleRow
Relu = mybir.ActivationFunctionType.Relu

_cached = {}


def _build():
    """Build + compile the SPMD NEFF (one core's program). Cached."""
    if "nc" in _cached:
        return _cached["nc"]

    nc = bacc.Bacc("TRN2", target_bir_lowering=False, debug=False,
                   num_devices=NCORES)

    # DRAM layouts are exact SBUF images (partition-major), packed host-side.
    x8_d = nc.dram_tensor("x8", [P, JB * 2 * NLOC * HW], FP8,
                          kind="ExternalInput")
    xr_d = nc.dram_tensor("xr", [NPAIRS, P, MB3 * NF], BF16,
                          kind="ExternalInput")
    w1_d = nc.dram_tensor("w1t", [P, JB * 2 * WIDTH], FP8,
                          kind="ExternalInput")
    w2_d = nc.dram_tensor("w2t", [P, 9 * 2 * WIDTH], FP8,
                          kind="ExternalInput")
    w3_d = nc.dram_tensor("w3t", [P, 2 * C_OUT], FP8, kind="ExternalInput")
    b_d = nc.dram_tensor("biases", [P, 4], F32, kind="ExternalInput")
    id_d = nc.dram_tensor("ident", [P, P], BF16, kind="ExternalInput")
    y_d = nc.dram_tensor("y", [NPAIRS, P, MB3 * NF], BF16,
                         kind="ExternalOutput")

    with tile.TileContext(nc) as tc:
        _emit(tc, nc, x8_d, xr_d, w1_d, w2_d, w3_d, b_d, id_d, y_d)

    nc.compile()
    _cached["nc"] = nc
    return nc


def _emit(tc, nc, x8_d, xr_d, w1_d, w2_d, w3_d, b_d, id_d, y_d):
    import contextlib

    Alu = mybir.AluOpType

    from concourse.tile import add_dep_helper

    with contextlib.ExitStack() as ctx:
        # One SBUF pool (per-tag bufs) + one PSUM pool: every pool adds
        # per-engine drain barriers to the kernel prologue/epilogue.
        sb = ctx.enter_context(tc.tile_pool(name="sb", bufs=1))
        const = xpool = opool = evp = sb
        # PSUM as 4 pair-tiles of [P, 2, 512] f32 (2 banks each = all 8
        # banks): two accumulation groups per tile in separate zero
        # regions, letting bias-free eviction pairs merge into one op.
        psp = ctx.enter_context(tc.tile_pool(name="psp", bufs=4, space="PSUM"))

        # ---- Loads -----------------------------------------------------
        # All bulk transfers as 2D [P, contiguous] DMAs on the two HWDGE
        # rings (sync + scalar): SWDGE (gpsimd) descriptor generation is a
        # software loop that delays first-byte by many microseconds.
        # Within a ring, DMAs execute FIFO. x8 gets the sync ring to
        # itself at full bandwidth (it gates conv1); w1+biases lead the
        # scalar ring (needed by the first matmul), while w2/w3/id are
        # dep-gated behind most of x8 so they don't steal bandwidth.
        x8sb = xpool.tile([P, JB, 2, NLOC * HW], FP8, name="x8sb", tag="x8sb")
        x8v = x8sb[:].rearrange("p j k f -> p (j k) f")
        x8src = x8_d.ap().rearrange("p (b f) -> p b f", b=2 * JB)

        w1sb = const.tile([P, JB, 2, WIDTH], FP8, name="w1sb", tag="w1sb")
        nc.scalar.dma_start(w1sb[:].rearrange("p a k c -> p (a k c)"),
                            w1_d.ap())

        ball = const.tile([P, 4], F32, name="ball", tag="ball")
        nc.scalar.dma_start(ball[:], b_d.ap())

        x_dmas = []
        for b in range(2 * JB):
            i = nc.sync.dma_start(x8v[:, b, :], x8src[:, b, :])
            x_dmas.append(i.ins)

        xrsb = xpool.tile([P, NPAIRS, MB3, NF], BF16, name="xrsb", tag="xrsb")
        for np_ in range(NPAIRS):
            dst = xrsb[:, np_, :, :].rearrange("p m f -> p (m f)")
            nc.sync.dma_start(dst, xr_d.ap()[np_])

        w2sb = const.tile([P, 9, 2, WIDTH], FP8, name="w2sb", tag="w2sb")
        i = nc.scalar.dma_start(w2sb[:].rearrange("p t k c -> p (t k c)"),
                                w2_d.ap())
        add_dep_helper(i.ins, x_dmas[5], reason="w2 after most of x8")

        w3sb = const.tile([P, 2, C_OUT], FP8, name="w3sb", tag="w3sb")
        nc.scalar.dma_start(w3sb[:].rearrange("p k c -> p (k c)"),
                            w3_d.ap())

        id_t = const.tile([P, P], BF16, name="id_t", tag="id_t")
        nc.scalar.dma_start(id_t[:], id_d.ap())

        # PE warm-up: bridge PE activity from body-start until the first x8
        # block pair lands (HAM keeps warming through conv1's own matmuls).
        scratch = const.tile([P, 512], BF16, name="scratch", tag="scratch")
        nc.gpsimd.memset(scratch[:], 0.0)
        warm_ps = psp.tile([P, 2, 512], F32, name="warm_ps", tag="pp")
        for _ in range(3):
            nc.tensor.matmul(warm_ps[:, 0, :], scratch[:, 0:P], scratch[:],
                             start=True, stop=True)

        # Zero-padded conv1 output: per image a 16x16 field per 128-block,
        # payload at rows/cols 1..14. Border zeroing on the otherwise-idle
        # GpSimd so DVE stays free for evictions.
        out1 = opool.tile([P, 2, NLOC, 16, 16], FP8, name="out1", tag="out1")
        o1flat = out1[:].rearrange("p k i r c -> p k (i r c)")
        for half in range(2):
            nc.gpsimd.memset(o1flat[:, :, half * 2048:(half + 1) * 2048], 0.0)

        out2 = opool.tile([P, 2, NLOC * HW], FP8, name="out2", tag="out2")

        tog = [0]

        def evict_relu_bias(dst, src, bias_ap):
            # dst = relu(src + bias), alternating DVE / ACT
            tog[0] ^= 1
            if tog[0]:
                nc.vector.tensor_scalar(dst, src, bias_ap, 0.0, Alu.add,
                                        Alu.max)
            else:
                nc.scalar.activation(dst, src, Relu, bias=bias_ap)

        # ---- conv1 (1x1, 1024->256) + bias + relu -> padded out1 --------
        # Per half: 8 open groups (4 pair-tiles x 2 out-blocks),
        # contraction j outer, groups inner so consecutive matmuls hit
        # different banks.
        for half in range(2):
            nls = range(4)
            pair = {nl: psp.tile([P, 2, 512], F32, name=f"ps1_{nl}",
                                 tag="pp") for nl in nls}
            for j in range(JB):
                for mo in range(2):
                    w_ap = w1sb[:, j, :, mo * P:(mo + 1) * P]
                    for nl in nls:
                        np_ = 4 * half + nl
                        nc.tensor.matmul(
                            pair[nl][:, mo, 0:NF], w_ap,
                            x8sb[:, j, :, np_ * NF:(np_ + 1) * NF],
                            start=(j == 0), stop=(j == JB - 1),
                            perf_mode=DR)
            for nl in nls:
                np_ = 4 * half + nl
                for mo in range(2):
                    dst = out1[:, mo, 2 * np_:2 * np_ + 2, 1:15, 1:15]
                    src = (pair[nl][:, mo, 0:NF]
                           .rearrange("p (i r c) -> p i r c", i=2, r=14))
                    evict_relu_bias(dst, src, ball[:, mo:mo + 1])

        # ---- conv2 + conv3, software-pipelined across super-blocks ------
        # Emission order c2(0), c2(1), c3(0), c2(2), c3(1), ... puts a full
        # conv2 block between conv3(s)'s evictions and the reuse of its
        # PSUM slots, removing the super-boundary bank-recycle stall.
        def emit_conv2(s):
            # conv2 (3x3, 256->256, pad 1): 8 groups (4 img pair-tiles x 2
            # out-blocks), contraction tap outer. Per-image matmuls (N=196):
            # the windowed moving AP [p, kt, r, c] is at the 3-free-dim ISA
            # limit. Evictions stay per-group (bias differs per out-block).
            p2 = {ii: psp.tile([P, 2, 512], F32, name=f"ps2_{ii}", tag="pp")
                  for ii in range(4)}
            for tap in range(9):
                dy, dx = tap // 3, tap % 3
                for mo in range(2):
                    w_ap = w2sb[:, tap, :, mo * P:(mo + 1) * P]
                    for ii in range(4):
                        img = 4 * s + ii
                        nc.tensor.matmul(
                            p2[ii][:, mo, 0:HW].rearrange("p (r c) -> p r c",
                                                          r=14),
                            w_ap,
                            out1[:, :, img, dy:dy + 14, dx:dx + 14],
                            start=(tap == 0), stop=(tap == 8),
                            perf_mode=DR)
            for ii in range(4):
                img = 4 * s + ii
                for mo in range(2):
                    evict_relu_bias(out2[:, mo, img * HW:(img + 1) * HW],
                                    p2[ii][:, mo, 0:HW], ball[:, 2 + mo:3 + mo])

        def emit_conv3(s):
            # conv3 (1x1, 256->1024) + residual + relu, two waves of 8
            # groups (4 m-blocks x 2 pairs). The residual (with bias3
            # pre-folded and pre-scaled, bf16) enters every group as an
            # identity matmul on the PE, so each eviction is a single
            # relu-cast op: eviction engines run ~0.56us per [P,392] op,
            # and halving their op count is worth more than the ~165ns/MM
            # the identity adds to the PE.
            yst = {nl: evp.tile([P, MB3 * NF], BF16, name=f"yst{nl}",
                                tag="yst", bufs=3) for nl in range(2)}
            for wave in range(2):
                p3 = {(mp, nl): psp.tile([P, 2, 512], F32,
                                         name=f"ps3_{mp}_{nl}", tag="pp")
                      for mp in range(2) for nl in range(2)}
                for mi in range(4):
                    m = 4 * wave + mi
                    w_ap = w3sb[:, :, m * P:(m + 1) * P]
                    for nl in range(2):
                        np_ = 2 * s + nl
                        nc.tensor.matmul(
                            p3[(mi // 2, nl)][:, mi % 2, 0:NF], w_ap,
                            out2[:, :, np_ * NF:(np_ + 1) * NF],
                            start=True, stop=False, perf_mode=DR)
                for mi in range(4):
                    m = 4 * wave + mi
                    for nl in range(2):
                        np_ = 2 * s + nl
                        nc.tensor.matmul(
                            p3[(mi // 2, nl)][:, mi % 2, 0:NF], id_t[:],
                            xrsb[:, np_, m, :], start=False, stop=True)
                # merged eviction: one relu-cast per pair-tile (bias3 is
                # already in the residual), writing two adjacent m-blocks
                for mp in range(2):
                    m0 = 4 * wave + 2 * mp
                    for nl in range(2):
                        dst = (yst[nl][:, m0 * NF:(m0 + 2) * NF]
                               .rearrange("p (g f) -> p g f", g=2))
                        src = p3[(mp, nl)][:, :, 0:NF]
                        tog[0] ^= 1
                        if tog[0]:
                            nc.vector.tensor_scalar_max(dst, src, 0.0)
                        else:
                            nc.scalar.activation(dst, src, Relu, bias=0.0)
                # per-wave y halves overlap the next wave's compute
                for nl in range(2):
                    np_ = 2 * s + nl
                    h0, h1 = 4 * wave * NF, (4 * wave + 4) * NF
                    nc.sync.dma_start(y_d.ap()[np_][:, h0:h1],
                                      yst[nl][:, h0:h1])

        emit_conv2(0)
        for s in range(1, 4):
            emit_conv2(s)
            emit_conv3(s - 1)
        emit_conv3(3)


def _prep(x, w1, g1, b1, m1, v1, w2, g2, b2, m2, v2, w3, g3, b3, m3, v3):
    """Host-side: fold BN, rescale + quantize to fp8, pack SBUF images."""
    def fold(w, g, b, m, v):
        scale = (g.astype(np.float64) / np.sqrt(v.astype(np.float64) + EPS))
        bias = b.astype(np.float64) - m.astype(np.float64) * scale
        wf = w.astype(np.float64) * scale.reshape(-1, *([1] * (w.ndim - 1)))
        return wf.astype(np.float32), bias.astype(np.float32)

    w1f, bias1 = fold(w1, g1, b1, m1, v1)   # [256,1024,1,1]
    w2f, bias2 = fold(w2, g2, b2, m2, v2)   # [256,256,3,3]
    w3f, bias3 = fold(w3, g3, b3, m3, v3)   # [1024,256,1,1]

    bf = ml_dtypes.bfloat16
    e4 = ml_dtypes.float8_e4m3

    def q8(a):
        return np.clip(a, -240.0, 240.0).astype(e4)

    # lhsT SBUF images [p_in, ..., ktile, co]:
    w1t = q8(np.ascontiguousarray(
        (w1f[:, :, 0, 0] * S1).T.reshape(JB, 2, P, WIDTH)
        .transpose(2, 0, 1, 3).reshape(P, JB * 2 * WIDTH)))
    w2t = q8(np.ascontiguousarray(
        (w2f * S2).transpose(2, 3, 1, 0).reshape(3, 3, 2, P, WIDTH)
        .transpose(3, 0, 1, 2, 4).reshape(P, 9 * 2 * WIDTH)))
    w3t = q8(np.ascontiguousarray(
        (w3f[:, :, 0, 0] * S3).T.reshape(2, P, C_OUT)
        .transpose(1, 0, 2).reshape(P, 2 * C_OUT)))

    b1h = (bias1 * S1).reshape(2, P).T                    # [P, 2]
    b2h = (bias2 * S1 * S2).reshape(2, P).T               # [P, 2] (64x)
    ball = np.ascontiguousarray(
        np.concatenate([b1h, b2h], axis=1), dtype=np.float32)

    # x8: conv1 moving operand, [core][P, (j, kt, img16, hw)] fp8
    xs = (x.reshape(NCORES, NLOC, JB, 2, P, HW)
          .transpose(0, 4, 2, 3, 1, 5)
          .reshape(NCORES, P, JB * 2 * NLOC * HW))
    x8 = q8(xs)

    # xr: residual + bias3, pre-scaled: STOT*(x + b3), np-major bf16
    r = x.reshape(NCORES, NLOC, C_OUT, HW) + bias3[None, None, :, None]
    xr = ((r * STOT)
          .reshape(NCORES, NPAIRS, 2, MB3, P, HW)
          .transpose(0, 1, 4, 3, 2, 5)
          .reshape(NCORES, NPAIRS, P, MB3 * NF)).astype(bf)

    common = {"w1t": w1t, "w2t": w2t, "w3t": w3t, "biases": ball,
              "ident": np.eye(P, dtype=np.float32).astype(bf)}
    in_maps = [dict(common, x8=np.ascontiguousarray(x8[i]),
                    xr=np.ascontiguousarray(xr[i]))
               for i in range(NCORES)]
    return in_maps


def kernel(**inputs):
    inputs = {k: np.asarray(v) for k, v in inputs.items()}
    in_maps = _prep(**inputs)
    nc = _build()
    res = run_bass_kernel_spmd(nc, in_maps, core_ids=list(range(NCORES)))

    y = np.empty((NCORES * NLOC, C_OUT, 14, 14), dtype=np.float32)
    for i in range(NCORES):
        r = np.asarray(res.results[i]["y"], dtype=np.float32) / STOT
        r = (r.reshape(NPAIRS, P, MB3, 2, HW)
             .transpose(0, 3, 2, 1, 4)
             .reshape(NLOC, C_OUT, 14, 14))
        y[i * NLOC:(i + 1) * NLOC] = r
    return y
